# revision 30
# baseline (speedup 1.0000x reference)
"""Trainium2 Bass kernel for nn_CrossLocal (cross-attention + convs + BN +
bilinear resizes), fused into a SINGLE launch across 8 NeuronCores.

Sharding: core = 2*b + qh owns half the query rows of batch b's 64x64 cross
image (32 rows = 2048 queries).  For qh=1 the row axis is mirrored on the
host so every core runs the identical SPMD program.  The single upsample
halo row each core needs (the partner's edge row) is computed on the host
(host prep already does the 1x1-conv projections) and shipped as an input,
so no halo ever rides a collective and no PE cycles are spent on it.

Host precomputes: y = theta_w@cr (keys), x = g_w@cr (queries),
z = phi(avgpool2x2(main)) row-scaled by e^r where r[m] = g_b.y_m is the
softmax-invariant bias, with an extra e^r column that accumulates the
softmax denominator for free.  Keys are PERMUTED: sorted by |y_m| and
interleaved low/high so that even-position key tiles (low score magnitude,
never softmax-dominant) can take an approximate exponential.

Device pipeline per core, per query superchunk J (1024 q) x key tile t:
  S[t] = y_t^T @ x_J                (2 bf16 matmuls, 4x PE row-tiling)
  es   = exp(S)                     split across TWO engines:
           odd tiles  -> ScalarE exact exp        (table spline)
           even tiles -> VectorE Schraudolph exp: int16(A*s+B) bitcast as
                         bf16 (~+-3% which is harmless on low-norm keys)
  acc += z_t^T @ es                 (K=128 bf16 matmuls into PSUM)
The PE matmul stream is the wall-clock: S pairs are emitted adjacently so
their row-quarter tiles overlap, and each az rides two tiles behind its
exp so the PE FIFO never head-of-line stalls on an in-flight exp (a >3.4us
PE bubble re-throttles the HAM clock gate to 1.2 GHz).

Per-J epilogue (h = (W@num + b*den)/den, BN stats) is cut into small steps
drained one-per-tile-pair inside the NEXT J's loop for the same reason.
BN statistics ride two tiny [64,2] AllGathers: AR#1 (J0 stats) overlaps
the J1 loop; AR#2 (J1 stats) is the only collective on the tail, hidden
behind the 2x bilinear upsample.  BN affine is folded into the output
epilogue (s*u + t + main), fp16 I/O throughout.
"""

import math
import os
from contextlib import ExitStack

import numpy as np
import ml_dtypes

import concourse.bass as bass
import concourse.tile as tile
from concourse import bacc, mybir
from concourse.bass_utils import run_bass_kernel_spmd

f32 = mybir.dt.float32
f32r = mybir.dt.float32r
fp16 = mybir.dt.float16
bf16 = mybir.dt.bfloat16
i16 = mybir.dt.int16
AF = mybir.ActivationFunctionType
ALU = mybir.AluOpType

NPBF16 = ml_dtypes.bfloat16

B, C, CI = 4, 64, 32
HM, HC = 128, 64
NC = HC * HC          # 4096 keys
NQ = 2048             # owned queries per core
NH = 64               # halo row width (host-computed)
NT = NC // 128        # 32 key tiles
BN_EPS = 1e-5
NSTAT = float(B * NC)
CORES = list(range(8))

# Schraudolph exp: es = bitcast_bf16(int16(EXPA*s + EXPB))
EXPA = 128.0 / math.log(2.0)
EXPB = 16256.0 - 0.058 * 128.0 + 0.5   # -c*128 minimax bias, +0.5 rounding

# engine split: tile t is VectorE-approx iff t even and t//2 < NDVE[J]
NDVE = (15, 14)

_cache = {}
last_profile = {}
_run_counter = [0]


def _run(nc, in_maps, name):
    trace = os.environ.get("BASS_PROFILE", "") == "1"
    tmpdir = None
    if trace:
        _run_counter[0] += 1
        tmpdir = os.path.join("/tmp/bass_traces",
                              f"{name}_{_run_counter[0]}")
        os.makedirs(tmpdir, exist_ok=True)
    br = run_bass_kernel_spmd(
        nc, in_maps, core_ids=CORES, trace=trace, tmpdir=tmpdir
    )
    if trace:
        last_profile[name] = br
    return br.results


def _build():
    nc = bacc.Bacc("TRN2", target_bir_lowering=False, debug=False,
                   num_devices=len(CORES))
    d_y1 = nc.dram_tensor("y1", [32, NC], bf16, kind="ExternalInput").ap()
    d_x1 = nc.dram_tensor("x1", [32, NQ], bf16, kind="ExternalInput").ap()
    d_z = nc.dram_tensor("zaug", [128, NT * 33], bf16, kind="ExternalInput").ap()
    d_wwt = nc.dram_tensor("wwt", [33, C], f32, kind="ExternalInput").ap()
    d_ones = nc.dram_tensor("ones64", [1, C], f32, kind="ExternalInput").ap()
    d_hh = nc.dram_tensor("hhalo", [C, NH], fp16, kind="ExternalInput").ap()
    d_g16 = nc.dram_tensor("g16", [128, 1], f32, kind="ExternalInput").ap()
    d_bet = nc.dram_tensor("bet", [128, 1], f32, kind="ExternalInput").ap()
    d_o = nc.dram_tensor("outp", [128, 32 * HM], fp16, kind="ExternalOutput").ap()

    with ExitStack() as ctx:
        tc = ctx.enter_context(tile.TileContext(nc))
        const = ctx.enter_context(tc.tile_pool(name="const", bufs=1))
        work = ctx.enter_context(tc.tile_pool(name="work", bufs=1))
        es_s = ctx.enter_context(tc.tile_pool(name="es_s", bufs=5))
        ps_s = ctx.enter_context(tc.tile_pool(name="ps_s", bufs=3, space="PSUM"))
        ps_fix = ctx.enter_context(tc.tile_pool(name="ps_fix", bufs=1, space="PSUM"))
        dram = ctx.enter_context(tc.tile_pool(name="dram", bufs=1, space="DRAM"))

        # --- warm the exp table ASAP ---
        warm = const.tile([1, 1], f32, tag="warm")
        nc.vector.memset(warm, 0.0)
        nc.scalar.activation(out=warm, in_=warm, func=AF.Exp)

        wsrc = const.tile([128, 512], bf16, tag="wsrc")
        nc.vector.memset(wsrc, 1.0)

        # --- constant loads: y/x ship un-replicated, quarters are
        # replicated on-device by SBUF-to-SBUF DMAs (4x less upload).
        # Replicas are emitted in fine chunks right behind the upload
        # chunk they mirror, so quarter-j tiles unblock progressively
        # instead of waiting for one big replica at the end of the queue.
        x4 = const.tile([128, NQ], bf16, tag="x4")
        nc.sync.dma_start(out=x4[0:32, 0:1024], in_=d_x1[:, 0:1024])
        for q in range(1, 4):
            nc.sync.dma_start(out=x4[32 * q:32 * q + 32, 0:1024],
                              in_=x4[0:32, 0:1024])
        y4 = const.tile([128, NC], bf16, tag="y4")
        for yc in range(4):
            sl = slice(1024 * yc, 1024 * yc + 1024)
            nc.sync.dma_start(out=y4[0:32, sl], in_=d_y1[:, sl])
            for q in range(1, 4):
                nc.sync.dma_start(out=y4[32 * q:32 * q + 32, sl],
                                  in_=y4[0:32, sl])
        nc.gpsimd.dma_start(out=x4[0:32, 1024:NQ], in_=d_x1[:, 1024:NQ])
        for q in range(1, 4):
            nc.gpsimd.dma_start(out=x4[32 * q:32 * q + 32, 1024:NQ],
                                in_=x4[0:32, 1024:NQ])
        z_sb = const.tile([128, NT, 33], bf16, tag="z_sb")
        nc.gpsimd.dma_start(out=z_sb, in_=d_z.rearrange("p (t s) -> p t s", s=33))
        wwt = const.tile([33, C], f32r, tag="wwt")
        nc.gpsimd.dma_start(out=wwt, in_=d_wwt)
        ones64 = const.tile([1, C], f32r, tag="ones64")
        nc.gpsimd.dma_start(out=ones64, in_=d_ones)
        g16 = const.tile([128, 1], f32, tag="g16")
        nc.sync.dma_start(out=g16, in_=d_g16)
        bet = const.tile([128, 1], f32, tag="bet")
        nc.sync.dma_start(out=bet, in_=d_bet)

        # --- persistent tiles ---
        h = work.tile([C, NQ], fp16, tag="h")
        junk = work.tile([C, 1024], fp16, tag="junk")
        sh = work.tile([C, 2], f32, tag="sh")
        sq = work.tile([C, 2], f32, tag="sq")
        acc = ps_fix.tile([128, 1024], f32, tag="acc")
        h2 = work.tile([128, 18, 66], fp16, tag="h2")
        # host-computed halo row lands straight in its upsample slot
        nc.gpsimd.dma_start(
            out=h2[C:128, 17:18, 1:65],
            in_=d_hh.rearrange("c (r w) -> c r w", w=64),
        )

        # DRAM bounce buffers for the two stats AllGathers
        ag1_in = dram.tile([C, 2], f32)
        ag1_out = dram.tile([8 * C, 2], f32)
        ag2_in = dram.tile([C, 2], f32)
        ag2_out = dram.tile([8 * C, 2], f32)

        def ham_filler(n):
            for wi in range(n):
                nc.tensor.matmul(
                    out=acc[0:C, 0:512], lhsT=wsrc[:, 0:C], rhs=wsrc,
                    start=(wi == 0), stop=(wi == n - 1),
                    skip_group_check=True,
                )

        ham_filler(10)

        def epilogue_steps(qoff, jj):
            """Emit-closures computing h[:, qoff:qoff+1024] from the az
            accumulator acc[64jj:64jj+33] (matmul scratch in acc's
            partitions 0:64, dead while the other J accumulates)."""
            accp = acc[64 * jj:64 * jj + 33, :]
            dbc_s, hp_s = acc[0:C, 0:512], acc[0:C, 512:1024]
            nsb = work.tile([33, 1024], f32r, tag="nsb", bufs=2)
            steps = [lambda: nc.vector.tensor_copy(out=nsb, in_=accp)]

            def chunk_a(c):
                nc.tensor.matmul(
                    out=dbc_s, lhsT=ones64, rhs=nsb[0:1, c:c + 512],
                    start=True, stop=True, skip_group_check=True,
                )
                rec = work.tile([C, 512], f32, tag="rec", bufs=2)
                nc.vector.reciprocal_approx_fast(out=rec, in_=dbc_s)
                return rec

            def chunk_b(c, rec):
                nc.tensor.matmul(
                    out=hp_s, lhsT=wwt, rhs=nsb[:, c:c + 512],
                    start=True, stop=True, skip_group_check=True,
                )
                nc.vector.tensor_tensor(
                    out=h[:, qoff + c:qoff + c + 512],
                    in0=hp_s, in1=rec, op=ALU.mult,
                )

            box = {}
            skip = lambda: None
            for c in (0, 512):
                # a full loop-pair of padding between each PE-bearing step
                # and its DVE-produced input, so the PE FIFO never stalls
                steps.append(skip)
                steps.append(lambda c=c: box.__setitem__(c, chunk_a(c)))
                steps.append(skip)
                steps.append(lambda c=c: chunk_b(c, box[c]))
            hv = h[:, qoff:qoff + 1024]
            steps.append(lambda: nc.vector.tensor_reduce(
                out=sh[:, jj:jj + 1], in_=hv,
                axis=mybir.AxisListType.X, op=ALU.add,
            ))
            steps.append(lambda: nc.scalar.activation(
                out=junk, in_=hv, func=AF.Square,
                accum_out=sq[:, jj:jj + 1],
            ))
            return steps

        def stats_trigger(jj, ag_in, ag_out):
            pay = work.tile([C, 2], f32, tag=f"pay{jj}")
            nc.gpsimd.tensor_copy(out=pay[:, 0:1], in_=sh[:, jj:jj + 1])
            nc.gpsimd.tensor_copy(out=pay[:, 1:2], in_=sq[:, jj:jj + 1])
            nc.gpsimd.dma_start(out=ag_in, in_=pay)
            nc.gpsimd.collective_compute(
                "AllGather", ALU.bypass,
                replica_groups=[CORES],
                ins=[ag_in[:].opt()], outs=[ag_out[:].opt()],
            )

        def stats_readback(jj, ag_out):
            g = work.tile([128, 8, 2], f32, tag=f"g{jj}")
            gv = ag_out.rearrange("(r c) s -> c r s", c=C)
            nc.sync.dma_start(out=g[0:C], in_=gv)
            nc.sync.dma_start(out=g[C:128], in_=gv)
            red = work.tile([128, 2], f32, tag=f"red{jj}")
            nc.vector.tensor_reduce(
                out=red, in_=g.rearrange("c r s -> c s r"),
                axis=mybir.AxisListType.X, op=ALU.add,
            )
            return red

        # ================= main attention loop =================
        # one flat stream of 64 tiles: the az pipeline (lag one pair) flows
        # straight across the J0/J1 boundary, so the PE never sees the
        # boundary drain bubble that re-throttled the HAM to half clock
        pending = []
        es_q = {}

        def emit_az(J, t):
            es = es_q.pop((J, t))
            for c in range(2):
                nc.tensor.matmul(
                    out=acc[64 * J:64 * J + 33, 512 * c:512 * c + 512],
                    lhsT=z_sb[:, t, :],
                    rhs=es[:, 512 * c:512 * c + 512],
                    start=(t == 0), stop=(t == NT - 1),
                    skip_group_check=True,
                    tile_position=(0, 64 * J),
                )

        for gp in range(NT):
            J, p = gp // 16, (gp % 16) * 2
            qoff = 1024 * J
            # S pair back-to-back: distinct row quarters overlap on PE
            sps = {}
            for t in (p, p + 1):
                # quarters 1-3 are on-device replicas that land a few
                # us in; the first tiles stick to quarter 0
                j = 0 if gp < 4 else t % 4
                s_ps = ps_s.tile([128, 1024], f32, tag="s")
                sps[t] = s_ps
                for c in range(2):
                    nc.tensor.matmul(
                        out=s_ps[:, 512 * c:512 * c + 512],
                        lhsT=y4[32 * j:32 * j + 32, 128 * t:128 * t + 128],
                        rhs=x4[32 * j:32 * j + 32,
                               qoff + 512 * c:qoff + 512 * c + 512],
                        start=True, stop=True,
                        skip_group_check=True,
                        tile_position=(32 * j, 0),
                    )
            # the previous pair's az rides behind this pair's S: deps
            # satisfied, the PE FIFO never stalls on an in-flight exp
            if gp >= 1:
                pJ, pp = (gp - 1) // 16, ((gp - 1) % 16) * 2
                emit_az(pJ, pp)
                emit_az(pJ, pp + 1)
                if pJ == 0 and pp == NT - 2:
                    # J0's accumulator just closed: queue its epilogue
                    pending = epilogue_steps(0, 0)
                    pending.append(lambda: stats_trigger(0, ag1_in, ag1_out))
                    pending.append(lambda: nc.sync.dma_start(
                        out=h2[0:C, 1:17, 1:65],
                        in_=h[:, 0:1024].rearrange("c (r w) -> c r w", w=64)))
                    pending.append(lambda: nc.sync.dma_start(
                        out=h2[0:C, 0:1, 1:65],
                        in_=h[:, 0:64].rearrange("c (r w) -> c r w", w=64)))
            for t in (p, p + 1):
                approx = (t % 2 == 0) and (t // 2 < NDVE[J])
                es = es_s.tile([128, 1024], bf16, tag="es")
                es_q[(J, t)] = es
                if approx:
                    nc.vector.tensor_scalar(
                        out=es.bitcast(i16), in0=sps[t],
                        scalar1=EXPA, scalar2=EXPB,
                        op0=ALU.mult, op1=ALU.add,
                    )
                else:
                    nc.scalar.activation(out=es, in_=sps[t], func=AF.Exp)
            if pending:
                pending.pop(0)()
        emit_az(1, NT - 2)
        emit_az(1, NT - 1)
        for step in pending:
            step()
        for step in epilogue_steps(1024, 1):
            step()
        # warm the sqrt table before the BN affine needs it
        nc.scalar.activation(out=warm, in_=warm, func=AF.Sqrt)
        stats_trigger(1, ag2_in, ag2_out)

        # --- h2 rest: row 16 + rowgroup 1 rows 15..31 ---
        nc.sync.dma_start(
            out=h2[0:C, 17:18, 1:65],
            in_=h[:, 1024:1088].rearrange("c (r w) -> c r w", w=64),
        )
        nc.sync.dma_start(
            out=h2[C:128, 0:17, 1:65],
            in_=h[:, 15 * 64:NQ].rearrange("c (r w) -> c r w", w=64),
        )
        # column clamp pads
        nc.vector.tensor_copy(out=h2[:, :, 0:1], in_=h2[:, :, 1:2])
        nc.vector.tensor_copy(out=h2[:, :, 65:66], in_=h2[:, :, 64:65])

        # --- 2x bilinear upsample (x16 scale folded into BN affine) ---
        rp = work.tile([128, 32, 66], fp16, tag="rp")
        nc.vector.scalar_tensor_tensor(
            out=rp[:, 0::2, :], in0=h2[:, 1:17, :], scalar=3.0,
            in1=h2[:, 0:16, :], op0=ALU.mult, op1=ALU.add,
        )
        nc.vector.scalar_tensor_tensor(
            out=rp[:, 1::2, :], in0=h2[:, 1:17, :], scalar=3.0,
            in1=h2[:, 2:18, :], op0=ALU.mult, op1=ALU.add,
        )
        u = work.tile([128, 32, HM], fp16, tag="u")
        nc.vector.scalar_tensor_tensor(
            out=u[:, :, 0::2], in0=rp[:, :, 1:65], scalar=3.0,
            in1=rp[:, :, 0:64], op0=ALU.mult, op1=ALU.add,
        )
        nc.vector.scalar_tensor_tensor(
            out=u[:, :, 1::2], in0=rp[:, :, 1:65], scalar=3.0,
            in1=rp[:, :, 2:66], op0=ALU.mult, op1=ALU.add,
        )

        # --- global BN stats -> affine (post AR#2) ---
        # late-hint everything AllGather-dependent so the scheduler keeps
        # it out of the busy mid-kernel engine FIFOs
        late = ctx.enter_context(tc.tile_wait_until(0.085))
        red1 = stats_readback(0, ag1_out)
        red2 = stats_readback(1, ag2_out)
        red = work.tile([128, 2], f32, tag="red")
        nc.vector.tensor_tensor(out=red, in0=red1, in1=red2, op=ALU.add)
        mom = work.tile([128, 2], f32, tag="mom")
        nc.vector.tensor_scalar(
            out=mom, in0=red, scalar1=1.0 / NSTAT, scalar2=None, op0=ALU.mult
        )
        msq = work.tile([128, 1], f32, tag="msq")
        nc.vector.tensor_tensor(
            out=msq, in0=mom[:, 0:1], in1=mom[:, 0:1], op=ALU.mult
        )
        varep = work.tile([128, 1], f32, tag="varep")
        nc.vector.scalar_tensor_tensor(
            out=varep, in0=mom[:, 1:2], scalar=1.0, in1=msq,
            op0=ALU.mult, op1=ALU.subtract,
        )
        nc.vector.tensor_scalar(
            out=varep, in0=varep, scalar1=BN_EPS, scalar2=None, op0=ALU.add
        )
        sd = work.tile([128, 1], f32, tag="sd")
        nc.scalar.activation(out=sd, in_=varep, func=AF.Sqrt)
        isd = work.tile([128, 1], f32, tag="isd")
        nc.vector.reciprocal(out=isd, in_=sd)
        s2 = work.tile([128, 1], f32, tag="s2")
        nc.vector.tensor_tensor(out=s2, in0=isd, in1=g16, op=ALU.mult)
        mt = work.tile([128, 1], f32, tag="mt")
        nc.vector.scalar_tensor_tensor(
            out=mt, in0=mom[:, 0:1], scalar=16.0, in1=s2,
            op0=ALU.mult, op1=ALU.mult,
        )
        t2 = work.tile([128, 1], f32, tag="t2")
        nc.vector.tensor_tensor(out=t2, in0=bet, in1=mt, op=ALU.subtract)

        # --- out = s*u + t (residual +main added on the host) ---
        bnu = work.tile([128, 32 * HM], fp16, tag="bnu")
        HF = 16 * HM
        for part in range(2):
            sl = slice(part * HF, part * HF + HF)
            nc.vector.tensor_scalar(
                out=bnu[:, sl], in0=u.rearrange("p r w -> p (r w)")[:, sl],
                scalar1=s2, scalar2=t2, op0=ALU.mult, op1=ALU.add,
            )
            nc.sync.dma_start(out=d_o[:, sl], in_=bnu[:, sl])

    nc.compile()
    return nc


def _tile_order():
    """Key-tile layout: tiles alternate low-norm (even slots) / high-norm
    (odd slots) so the VectorE-approx tiles interleave with ScalarE tiles
    in time.  Returns perm[t] = sorted-tile-rank placed at position t."""
    order = np.empty(NT, dtype=np.int64)
    order[0::2] = np.arange(16)          # low-norm halves
    order[1::2] = 16 + np.arange(16)     # high-norm halves
    return order


def _host_prep(main_feature, cross_feature, g_w, g_b, theta_w, theta_b,
               phi_w, phi_b, w_w, w_b, bn_gamma, bn_beta):
    per_batch = []
    tile_order = _tile_order()
    for b in range(B):
        cr = cross_feature[b].reshape(C, NC).astype(np.float64)
        y = theta_w.astype(np.float64) @ cr                 # [32, 4096]
        x = g_w.astype(np.float64) @ cr                     # [32, 4096]
        r = (g_b.astype(np.float64) @ y)                    # [4096]
        m = main_feature[b]
        pm = 0.25 * (m[:, 0::2, 0::2] + m[:, 0::2, 1::2]
                     + m[:, 1::2, 0::2] + m[:, 1::2, 1::2])
        z = phi_w.astype(np.float64) @ pm.reshape(C, NC).astype(np.float64)
        z = z + phi_b.astype(np.float64)[:, None]           # [32, 4096]

        # the two upsample halo rows (globals 31, 32) computed exactly
        xh = x.reshape(CI, HC, HC)[:, 31:33, :].reshape(CI, 2 * HC)
        sh_ = y.T @ xh + r[:, None]                         # [4096, 128]
        sh_ -= sh_.max(axis=0)
        esh = np.exp(sh_)
        att = (z @ esh) / esh.sum(axis=0)                   # [32, 128]
        hh = (w_w.astype(np.float64) @ att
              + w_b.astype(np.float64)[:, None])            # [64, 128]
        hh = hh.reshape(C, 2, HC).astype(np.float16)

        # permute keys: sort by |y| then interleave low/high norm tiles
        key_rank = np.argsort(np.linalg.norm(y, axis=0), kind="stable")
        key_perm = key_rank.reshape(NT, 128)[tile_order].reshape(-1)
        y = y[:, key_perm]
        z = z[:, key_perm]
        r = r[key_perm]

        y1 = y.astype(np.float32).astype(NPBF16)
        er = np.exp(r)
        zt = np.empty((128, NT, 33), dtype=np.float64)
        zt[:, :, 0] = 1.0
        zt[:, :, 1:] = z.T.reshape(NT, 128, CI).transpose(1, 0, 2)
        zt *= er.reshape(NT, 128).T[:, :, None]
        per_batch.append((x, y1, zt.astype(NPBF16), hh))

    wwt = np.concatenate([w_b[None, :], w_w.T], axis=0).astype(np.float32)
    ones64 = np.ones((1, C), dtype=np.float32)
    g16 = np.tile((bn_gamma / 16.0).astype(np.float32)[:, None], (2, 1))
    bet = np.tile(bn_beta.astype(np.float32)[:, None], (2, 1))

    in_maps = []
    for core in CORES:
        b, qh = core // 2, core % 2
        x, y1, zt, hh = per_batch[b]
        # local row l -> global row g: qh=0: g=l ; qh=1: g=63-l
        if qh == 0:
            rows = np.arange(32)
        else:
            rows = 63 - np.arange(32)
        xq = x.reshape(CI, HC, HC)[:, rows, :].reshape(CI, NQ)
        x1 = xq.astype(np.float32).astype(NPBF16)
        # halo: the row just past local row 31 (global 32 / 31)
        hhalo = np.ascontiguousarray(hh[:, 1 - qh, :])
        in_maps.append({
            "y1": y1, "x1": x1,
            "zaug": zt.reshape(128, NT * 33),
            "wwt": wwt, "ones64": ones64, "hhalo": hhalo,
            "g16": g16, "bet": bet,
        })
    return in_maps


def kernel(main_feature, cross_feature, g_w, g_b, theta_w, theta_b,
           phi_w, phi_b, w_w, w_b, bn_gamma, bn_beta):
    main_feature = np.ascontiguousarray(main_feature, dtype=np.float32)
    cross_feature = np.ascontiguousarray(cross_feature, dtype=np.float32)

    if "k" not in _cache:
        _cache["k"] = _build()

    in_maps = _host_prep(main_feature, cross_feature, g_w, g_b, theta_w,
                         theta_b, phi_w, phi_b, w_w, w_b, bn_gamma, bn_beta)
    res = _run(_cache["k"], in_maps, "k")

    out = np.empty((B, C, HM, HM), dtype=np.float32)
    for core in CORES:
        b, qh = core // 2, core % 2
        v = res[core]["outp"].astype(np.float32).reshape(2, C, 32, HM)
        v = v.transpose(1, 0, 2, 3).reshape(C, 64, HM)
        if qh == 0:
            out[b][:, 0:64, :] = v
        else:
            out[b][:, 64:128, :] = v[:, ::-1, :]
    out += main_feature
    return out


# revision 31
# speedup vs baseline: 1.1260x; 1.1260x over previous
"""Trainium2 Bass kernel for nn_CrossLocal (cross-attention + convs + BN +
bilinear resizes), fused into a SINGLE launch across 8 NeuronCores.

Sharding: core = 2*b + qh owns half the query rows of batch b's 64x64 cross
image (32 rows = 2048 queries).  For qh=1 the row axis is mirrored on the
host so every core runs the identical SPMD program.  The single upsample
halo row each core needs (the partner's edge row) is computed on the host
(host prep already does the 1x1-conv projections) and shipped as an input,
so no halo ever rides a collective and no PE cycles are spent on it.

Host precomputes: y = theta_w@cr (keys), x = g_w@cr (queries),
z = phi(avgpool2x2(main)) row-scaled by e^r where r[m] = g_b.y_m is the
softmax-invariant bias, with an extra e^r column that accumulates the
softmax denominator for free.  Keys are PERMUTED: sorted by |y_m| and
interleaved low/high so that even-position key tiles (low score magnitude,
never softmax-dominant) can take an approximate exponential.

Device pipeline per core, per query superchunk J (1024 q) x key tile t:
  S[t] = y_t^T @ x_J                (2 bf16 matmuls, 4x PE row-tiling)
  es   = exp(S)                     split across TWO engines:
           odd tiles  -> ScalarE exact exp        (table spline)
           even tiles -> VectorE Schraudolph exp: int16(A*s+B) bitcast as
                         bf16 (~+-3% which is harmless on low-norm keys)
  acc += z_t^T @ es                 (K=128 bf16 matmuls into PSUM)
The PE matmul stream is the wall-clock: S pairs are emitted adjacently so
their row-quarter tiles overlap, and each az rides two tiles behind its
exp so the PE FIFO never head-of-line stalls on an in-flight exp (a >3.4us
PE bubble re-throttles the HAM clock gate to 1.2 GHz).

Per-J epilogue (h = (W@num + b*den)/den, BN stats) is cut into small steps
drained one-per-tile-pair inside the NEXT J's loop for the same reason.
BN statistics ride two tiny [64,2] AllGathers: AR#1 (J0 stats) overlaps
the J1 loop; AR#2 (J1 stats) is the only collective on the tail, hidden
behind the 2x bilinear upsample.  BN affine is folded into the output
epilogue (s*u + t + main), fp16 I/O throughout.
"""

import math
import os
from contextlib import ExitStack

import numpy as np
import ml_dtypes

import concourse.bass as bass
import concourse.tile as tile
from concourse import bacc, mybir
from concourse.bass_utils import run_bass_kernel_spmd

f32 = mybir.dt.float32
f32r = mybir.dt.float32r
fp16 = mybir.dt.float16
bf16 = mybir.dt.bfloat16
i16 = mybir.dt.int16
AF = mybir.ActivationFunctionType
ALU = mybir.AluOpType

NPBF16 = ml_dtypes.bfloat16

B, C, CI = 4, 64, 32
HM, HC = 128, 64
NC = HC * HC          # 4096 keys
NQ = 2048             # owned queries per core
NH = 64               # halo row width (host-computed)
NT = NC // 128        # 32 key tiles
BN_EPS = 1e-5
NSTAT = float(B * NC)
CORES = list(range(8))

# Schraudolph exp: es = bitcast_bf16(int16(EXPA*s + EXPB))
EXPA = 128.0 / math.log(2.0)
EXPB = 16256.0 - 0.058 * 128.0 + 0.5   # -c*128 minimax bias, +0.5 rounding

# engine split: tile t is VectorE-approx iff t even and t//2 < NDVE[J]
# (J1 gives VectorE one more tile: ScalarE's last exp paces the final az
# matmul and the AR#2 trigger chain)
NDVE = (15, 15)

_cache = {}
last_profile = {}
_run_counter = [0]


def _run(nc, in_maps, name):
    trace = os.environ.get("BASS_PROFILE", "") == "1"
    tmpdir = None
    if trace:
        _run_counter[0] += 1
        tmpdir = os.path.join("/tmp/bass_traces",
                              f"{name}_{_run_counter[0]}")
        os.makedirs(tmpdir, exist_ok=True)
    br = run_bass_kernel_spmd(
        nc, in_maps, core_ids=CORES, trace=trace, tmpdir=tmpdir
    )
    if trace:
        last_profile[name] = br
    return br.results


def _build():
    nc = bacc.Bacc("TRN2", target_bir_lowering=False, debug=False,
                   num_devices=len(CORES))
    d_y1 = nc.dram_tensor("y1", [32, NC], bf16, kind="ExternalInput").ap()
    d_x1 = nc.dram_tensor("x1", [32, NQ], bf16, kind="ExternalInput").ap()
    d_z = nc.dram_tensor("zaug", [128, NT * 33], bf16, kind="ExternalInput").ap()
    d_wwt = nc.dram_tensor("wwt", [33, C], f32, kind="ExternalInput").ap()
    d_ones = nc.dram_tensor("ones64", [1, C], f32, kind="ExternalInput").ap()
    d_hh = nc.dram_tensor("hhalo", [C, NH], fp16, kind="ExternalInput").ap()
    d_g16 = nc.dram_tensor("g16", [128, 1], f32, kind="ExternalInput").ap()
    d_bet = nc.dram_tensor("bet", [128, 1], f32, kind="ExternalInput").ap()
    d_o = nc.dram_tensor("outp", [128, 32 * HM], fp16, kind="ExternalOutput").ap()

    with ExitStack() as ctx:
        tc = ctx.enter_context(tile.TileContext(nc))
        const = ctx.enter_context(tc.tile_pool(name="const", bufs=1))
        work = ctx.enter_context(tc.tile_pool(name="work", bufs=1))
        es_s = ctx.enter_context(tc.tile_pool(name="es_s", bufs=5))
        ps_s = ctx.enter_context(tc.tile_pool(name="ps_s", bufs=3, space="PSUM"))
        ps_fix = ctx.enter_context(tc.tile_pool(name="ps_fix", bufs=1, space="PSUM"))
        dram = ctx.enter_context(tc.tile_pool(name="dram", bufs=1, space="DRAM"))

        # --- warm the exp table ASAP ---
        warm = const.tile([1, 1], f32, tag="warm")
        nc.vector.memset(warm, 0.0)
        nc.scalar.activation(out=warm, in_=warm, func=AF.Exp)

        wsrc = const.tile([128, 512], bf16, tag="wsrc")
        nc.vector.memset(wsrc, 1.0)

        # --- constant loads: y/x ship un-replicated, quarters are
        # replicated on-device by SBUF-to-SBUF DMAs (4x less upload).
        # Replicas are emitted in fine chunks right behind the upload
        # chunk they mirror, so quarter-j tiles unblock progressively
        # instead of waiting for one big replica at the end of the queue.
        x4 = const.tile([128, NQ], bf16, tag="x4")
        nc.sync.dma_start(out=x4[0:32, 0:1024], in_=d_x1[:, 0:1024])
        for q in range(1, 4):
            nc.sync.dma_start(out=x4[32 * q:32 * q + 32, 0:1024],
                              in_=x4[0:32, 0:1024])
        y4 = const.tile([128, NC], bf16, tag="y4")
        for yc in range(4):
            sl = slice(1024 * yc, 1024 * yc + 1024)
            nc.sync.dma_start(out=y4[0:32, sl], in_=d_y1[:, sl])
            for q in range(1, 4):
                nc.sync.dma_start(out=y4[32 * q:32 * q + 32, sl],
                                  in_=y4[0:32, sl])
        nc.gpsimd.dma_start(out=x4[0:32, 1024:NQ], in_=d_x1[:, 1024:NQ])
        for q in range(1, 4):
            nc.gpsimd.dma_start(out=x4[32 * q:32 * q + 32, 1024:NQ],
                                in_=x4[0:32, 1024:NQ])
        z_sb = const.tile([128, NT, 33], bf16, tag="z_sb")
        nc.gpsimd.dma_start(out=z_sb, in_=d_z.rearrange("p (t s) -> p t s", s=33))
        wwt = const.tile([33, C], f32r, tag="wwt")
        nc.gpsimd.dma_start(out=wwt, in_=d_wwt)
        ones64 = const.tile([1, C], f32r, tag="ones64")
        nc.gpsimd.dma_start(out=ones64, in_=d_ones)
        g16 = const.tile([128, 1], f32, tag="g16")
        nc.sync.dma_start(out=g16, in_=d_g16)
        bet = const.tile([128, 1], f32, tag="bet")
        nc.sync.dma_start(out=bet, in_=d_bet)

        # --- persistent tiles ---
        h = work.tile([C, NQ], fp16, tag="h")
        junk = work.tile([C, 1024], fp16, tag="junk")
        sh = work.tile([C, 2], f32, tag="sh")
        sq = work.tile([C, 2], f32, tag="sq")
        acc = ps_fix.tile([128, 1024], f32, tag="acc")
        h2 = work.tile([128, 18, 66], fp16, tag="h2")
        # host-computed halo row lands straight in its upsample slot
        nc.gpsimd.dma_start(
            out=h2[C:128, 17:18, 1:65],
            in_=d_hh.rearrange("c (r w) -> c r w", w=64),
        )

        # DRAM bounce buffers for the two stats AllGathers
        ag1_in = dram.tile([C, 2], f32)
        ag1_out = dram.tile([8 * C, 2], f32)
        ag2_in = dram.tile([C, 2], f32)
        ag2_out = dram.tile([8 * C, 2], f32)

        def ham_filler(n):
            for wi in range(n):
                nc.tensor.matmul(
                    out=acc[0:C, 0:512], lhsT=wsrc[:, 0:C], rhs=wsrc,
                    start=(wi == 0), stop=(wi == n - 1),
                    skip_group_check=True,
                )

        ham_filler(10)

        def epilogue_steps(qoff, jj):
            """Emit-closures computing h[:, qoff:qoff+1024] from the az
            accumulator acc[64jj:64jj+33] (matmul scratch in acc's
            partitions 0:64, dead while the other J accumulates)."""
            accp = acc[64 * jj:64 * jj + 33, :]
            dbc_s, hp_s = acc[0:C, 0:512], acc[0:C, 512:1024]
            nsb = work.tile([33, 1024], f32r, tag="nsb", bufs=2)
            steps = [lambda: nc.vector.tensor_copy(out=nsb, in_=accp)]

            def chunk_a(c):
                nc.tensor.matmul(
                    out=dbc_s, lhsT=ones64, rhs=nsb[0:1, c:c + 512],
                    start=True, stop=True, skip_group_check=True,
                )
                rec = work.tile([C, 512], f32, tag="rec", bufs=2)
                nc.vector.reciprocal_approx_fast(out=rec, in_=dbc_s)
                return rec

            def chunk_b(c, rec):
                nc.tensor.matmul(
                    out=hp_s, lhsT=wwt, rhs=nsb[:, c:c + 512],
                    start=True, stop=True, skip_group_check=True,
                )
                nc.vector.tensor_tensor(
                    out=h[:, qoff + c:qoff + c + 512],
                    in0=hp_s, in1=rec, op=ALU.mult,
                )

            box = {}
            skip = lambda: None
            for c in (0, 512):
                # a full loop-pair of padding between each PE-bearing step
                # and its DVE-produced input, so the PE FIFO never stalls
                steps.append(skip)
                steps.append(lambda c=c: box.__setitem__(c, chunk_a(c)))
                steps.append(skip)
                steps.append(lambda c=c: chunk_b(c, box[c]))
            hv = h[:, qoff:qoff + 1024]
            steps.append(lambda: nc.vector.tensor_reduce(
                out=sh[:, jj:jj + 1], in_=hv,
                axis=mybir.AxisListType.X, op=ALU.add,
            ))
            steps.append(lambda: nc.scalar.activation(
                out=junk, in_=hv, func=AF.Square,
                accum_out=sq[:, jj:jj + 1],
            ))
            return steps

        def stats_trigger(jj, ag_in, ag_out):
            pay = work.tile([C, 2], f32, tag=f"pay{jj}")
            nc.gpsimd.tensor_copy(out=pay[:, 0:1], in_=sh[:, jj:jj + 1])
            nc.gpsimd.tensor_copy(out=pay[:, 1:2], in_=sq[:, jj:jj + 1])
            nc.gpsimd.dma_start(out=ag_in, in_=pay)
            nc.gpsimd.collective_compute(
                "AllGather", ALU.bypass,
                replica_groups=[CORES],
                ins=[ag_in[:].opt()], outs=[ag_out[:].opt()],
            )

        def stats_readback(jj, ag_out):
            g = work.tile([128, 8, 2], f32, tag=f"g{jj}")
            gv = ag_out.rearrange("(r c) s -> c r s", c=C)
            nc.sync.dma_start(out=g[0:C], in_=gv)
            nc.sync.dma_start(out=g[C:128], in_=gv)
            red = work.tile([128, 2], f32, tag=f"red{jj}")
            nc.vector.tensor_reduce(
                out=red, in_=g.rearrange("c r s -> c s r"),
                axis=mybir.AxisListType.X, op=ALU.add,
            )
            return red

        # ================= main attention loop =================
        # one flat stream of 64 tiles: the az pipeline (lag one pair) flows
        # straight across the J0/J1 boundary, so the PE never sees the
        # boundary drain bubble that re-throttled the HAM to half clock
        pending = []
        es_q = {}

        def emit_az(J, t):
            es = es_q.pop((J, t))
            for c in range(2):
                nc.tensor.matmul(
                    out=acc[64 * J:64 * J + 33, 512 * c:512 * c + 512],
                    lhsT=z_sb[:, t, :],
                    rhs=es[:, 512 * c:512 * c + 512],
                    start=(t == 0), stop=(t == NT - 1),
                    skip_group_check=True,
                    tile_position=(0, 64 * J),
                )

        for gp in range(NT):
            J, p = gp // 16, (gp % 16) * 2
            qoff = 1024 * J
            # S pair back-to-back: distinct row quarters overlap on PE
            sps = {}
            for t in (p, p + 1):
                # quarters 1-3 are on-device replicas that land a few
                # us in; the first tiles stick to quarter 0
                j = 0 if gp < 4 else t % 4
                s_ps = ps_s.tile([128, 1024], f32, tag="s")
                sps[t] = s_ps
                for c in range(2):
                    nc.tensor.matmul(
                        out=s_ps[:, 512 * c:512 * c + 512],
                        lhsT=y4[32 * j:32 * j + 32, 128 * t:128 * t + 128],
                        rhs=x4[32 * j:32 * j + 32,
                               qoff + 512 * c:qoff + 512 * c + 512],
                        start=True, stop=True,
                        skip_group_check=True,
                        tile_position=(32 * j, 0),
                    )
            # the previous pair's az rides behind this pair's S: deps
            # satisfied, the PE FIFO never stalls on an in-flight exp
            if gp >= 1:
                pJ, pp = (gp - 1) // 16, ((gp - 1) % 16) * 2
                emit_az(pJ, pp)
                emit_az(pJ, pp + 1)
                if pJ == 0 and pp == NT - 2:
                    # J0's accumulator just closed: queue its epilogue
                    pending = epilogue_steps(0, 0)
                    pending.append(lambda: stats_trigger(0, ag1_in, ag1_out))
                    pending.append(lambda: nc.sync.dma_start(
                        out=h2[0:C, 1:17, 1:65],
                        in_=h[:, 0:1024].rearrange("c (r w) -> c r w", w=64)))
                    pending.append(lambda: nc.sync.dma_start(
                        out=h2[0:C, 0:1, 1:65],
                        in_=h[:, 0:64].rearrange("c (r w) -> c r w", w=64)))
            for t in (p, p + 1):
                approx = (t % 2 == 0) and (t // 2 < NDVE[J])
                es = es_s.tile([128, 1024], bf16, tag="es")
                es_q[(J, t)] = es
                if approx:
                    nc.vector.tensor_scalar(
                        out=es.bitcast(i16), in0=sps[t],
                        scalar1=EXPA, scalar2=EXPB,
                        op0=ALU.mult, op1=ALU.add,
                    )
                else:
                    nc.scalar.activation(out=es, in_=sps[t], func=AF.Exp)
            if pending:
                pending.pop(0)()
        emit_az(1, NT - 2)
        emit_az(1, NT - 1)
        for step in pending:
            step()
        for step in epilogue_steps(1024, 1):
            step()
        # warm the sqrt table before the BN affine needs it
        nc.scalar.activation(out=warm, in_=warm, func=AF.Sqrt)
        stats_trigger(1, ag2_in, ag2_out)

        # --- h2 rest: row 16 + rowgroup 1 rows 15..31 ---
        nc.sync.dma_start(
            out=h2[0:C, 17:18, 1:65],
            in_=h[:, 1024:1088].rearrange("c (r w) -> c r w", w=64),
        )
        nc.sync.dma_start(
            out=h2[C:128, 0:17, 1:65],
            in_=h[:, 15 * 64:NQ].rearrange("c (r w) -> c r w", w=64),
        )
        # column clamp pads
        nc.vector.tensor_copy(out=h2[:, :, 0:1], in_=h2[:, :, 1:2])
        nc.vector.tensor_copy(out=h2[:, :, 65:66], in_=h2[:, :, 64:65])

        # --- 2x bilinear upsample (x16 scale folded into BN affine) ---
        rp = work.tile([128, 32, 66], fp16, tag="rp")
        nc.vector.scalar_tensor_tensor(
            out=rp[:, 0::2, :], in0=h2[:, 1:17, :], scalar=3.0,
            in1=h2[:, 0:16, :], op0=ALU.mult, op1=ALU.add,
        )
        nc.vector.scalar_tensor_tensor(
            out=rp[:, 1::2, :], in0=h2[:, 1:17, :], scalar=3.0,
            in1=h2[:, 2:18, :], op0=ALU.mult, op1=ALU.add,
        )
        u = work.tile([128, 32, HM], fp16, tag="u")
        nc.vector.scalar_tensor_tensor(
            out=u[:, :, 0::2], in0=rp[:, :, 1:65], scalar=3.0,
            in1=rp[:, :, 0:64], op0=ALU.mult, op1=ALU.add,
        )
        nc.vector.scalar_tensor_tensor(
            out=u[:, :, 1::2], in0=rp[:, :, 1:65], scalar=3.0,
            in1=rp[:, :, 2:66], op0=ALU.mult, op1=ALU.add,
        )

        # --- global BN stats -> affine (post AR#2) ---
        # late-hint everything AllGather-dependent so the scheduler keeps
        # it out of the busy mid-kernel engine FIFOs
        late = ctx.enter_context(tc.tile_wait_until(0.085))
        red1 = stats_readback(0, ag1_out)
        red2 = stats_readback(1, ag2_out)
        red = work.tile([128, 2], f32, tag="red")
        nc.vector.tensor_tensor(out=red, in0=red1, in1=red2, op=ALU.add)
        mom = work.tile([128, 2], f32, tag="mom")
        nc.vector.tensor_scalar(
            out=mom, in0=red, scalar1=1.0 / NSTAT, scalar2=None, op0=ALU.mult
        )
        msq = work.tile([128, 1], f32, tag="msq")
        nc.vector.tensor_tensor(
            out=msq, in0=mom[:, 0:1], in1=mom[:, 0:1], op=ALU.mult
        )
        varep = work.tile([128, 1], f32, tag="varep")
        nc.vector.scalar_tensor_tensor(
            out=varep, in0=mom[:, 1:2], scalar=1.0, in1=msq,
            op0=ALU.mult, op1=ALU.subtract,
        )
        nc.vector.tensor_scalar(
            out=varep, in0=varep, scalar1=BN_EPS, scalar2=None, op0=ALU.add
        )
        sd = work.tile([128, 1], f32, tag="sd")
        nc.scalar.activation(out=sd, in_=varep, func=AF.Sqrt)
        isd = work.tile([128, 1], f32, tag="isd")
        nc.vector.reciprocal(out=isd, in_=sd)
        s2 = work.tile([128, 1], f32, tag="s2")
        nc.vector.tensor_tensor(out=s2, in0=isd, in1=g16, op=ALU.mult)
        mt = work.tile([128, 1], f32, tag="mt")
        nc.vector.scalar_tensor_tensor(
            out=mt, in0=mom[:, 0:1], scalar=16.0, in1=s2,
            op0=ALU.mult, op1=ALU.mult,
        )
        t2 = work.tile([128, 1], f32, tag="t2")
        nc.vector.tensor_tensor(out=t2, in0=bet, in1=mt, op=ALU.subtract)

        # --- out = s*u + t (residual +main added on the host) ---
        bnu = work.tile([128, 32 * HM], fp16, tag="bnu")
        HF = 16 * HM
        for part in range(2):
            sl = slice(part * HF, part * HF + HF)
            nc.vector.tensor_scalar(
                out=bnu[:, sl], in0=u.rearrange("p r w -> p (r w)")[:, sl],
                scalar1=s2, scalar2=t2, op0=ALU.mult, op1=ALU.add,
            )
            nc.sync.dma_start(out=d_o[:, sl], in_=bnu[:, sl])

    nc.compile()
    return nc


def _tile_order():
    """Key-tile layout: tiles alternate low-norm (even slots) / high-norm
    (odd slots) so the VectorE-approx tiles interleave with ScalarE tiles
    in time.  Returns perm[t] = sorted-tile-rank placed at position t."""
    order = np.empty(NT, dtype=np.int64)
    order[0::2] = np.arange(16)          # low-norm halves
    order[1::2] = 16 + np.arange(16)     # high-norm halves
    return order


def _host_prep(main_feature, cross_feature, g_w, g_b, theta_w, theta_b,
               phi_w, phi_b, w_w, w_b, bn_gamma, bn_beta):
    per_batch = []
    tile_order = _tile_order()
    for b in range(B):
        cr = cross_feature[b].reshape(C, NC).astype(np.float64)
        y = theta_w.astype(np.float64) @ cr                 # [32, 4096]
        x = g_w.astype(np.float64) @ cr                     # [32, 4096]
        r = (g_b.astype(np.float64) @ y)                    # [4096]
        m = main_feature[b]
        pm = 0.25 * (m[:, 0::2, 0::2] + m[:, 0::2, 1::2]
                     + m[:, 1::2, 0::2] + m[:, 1::2, 1::2])
        z = phi_w.astype(np.float64) @ pm.reshape(C, NC).astype(np.float64)
        z = z + phi_b.astype(np.float64)[:, None]           # [32, 4096]

        # the two upsample halo rows (globals 31, 32) computed exactly
        xh = x.reshape(CI, HC, HC)[:, 31:33, :].reshape(CI, 2 * HC)
        sh_ = y.T @ xh + r[:, None]                         # [4096, 128]
        sh_ -= sh_.max(axis=0)
        esh = np.exp(sh_)
        att = (z @ esh) / esh.sum(axis=0)                   # [32, 128]
        hh = (w_w.astype(np.float64) @ att
              + w_b.astype(np.float64)[:, None])            # [64, 128]
        hh = hh.reshape(C, 2, HC).astype(np.float16)

        # permute keys: sort by |y| then interleave low/high norm tiles
        key_rank = np.argsort(np.linalg.norm(y, axis=0), kind="stable")
        key_perm = key_rank.reshape(NT, 128)[tile_order].reshape(-1)
        y = y[:, key_perm]
        z = z[:, key_perm]
        r = r[key_perm]

        y1 = y.astype(np.float32).astype(NPBF16)
        er = np.exp(r)
        zt = np.empty((128, NT, 33), dtype=np.float64)
        zt[:, :, 0] = 1.0
        zt[:, :, 1:] = z.T.reshape(NT, 128, CI).transpose(1, 0, 2)
        zt *= er.reshape(NT, 128).T[:, :, None]
        per_batch.append((x, y1, zt.astype(NPBF16), hh))

    wwt = np.concatenate([w_b[None, :], w_w.T], axis=0).astype(np.float32)
    ones64 = np.ones((1, C), dtype=np.float32)
    g16 = np.tile((bn_gamma / 16.0).astype(np.float32)[:, None], (2, 1))
    bet = np.tile(bn_beta.astype(np.float32)[:, None], (2, 1))

    in_maps = []
    for core in CORES:
        b, qh = core // 2, core % 2
        x, y1, zt, hh = per_batch[b]
        # local row l -> global row g: qh=0: g=l ; qh=1: g=63-l
        if qh == 0:
            rows = np.arange(32)
        else:
            rows = 63 - np.arange(32)
        xq = x.reshape(CI, HC, HC)[:, rows, :].reshape(CI, NQ)
        x1 = xq.astype(np.float32).astype(NPBF16)
        # halo: the row just past local row 31 (global 32 / 31)
        hhalo = np.ascontiguousarray(hh[:, 1 - qh, :])
        in_maps.append({
            "y1": y1, "x1": x1,
            "zaug": zt.reshape(128, NT * 33),
            "wwt": wwt, "ones64": ones64, "hhalo": hhalo,
            "g16": g16, "bet": bet,
        })
    return in_maps


def kernel(main_feature, cross_feature, g_w, g_b, theta_w, theta_b,
           phi_w, phi_b, w_w, w_b, bn_gamma, bn_beta):
    main_feature = np.ascontiguousarray(main_feature, dtype=np.float32)
    cross_feature = np.ascontiguousarray(cross_feature, dtype=np.float32)

    if "k" not in _cache:
        _cache["k"] = _build()

    in_maps = _host_prep(main_feature, cross_feature, g_w, g_b, theta_w,
                         theta_b, phi_w, phi_b, w_w, w_b, bn_gamma, bn_beta)
    res = _run(_cache["k"], in_maps, "k")

    out = np.empty((B, C, HM, HM), dtype=np.float32)
    for core in CORES:
        b, qh = core // 2, core % 2
        v = res[core]["outp"].astype(np.float32).reshape(2, C, 32, HM)
        v = v.transpose(1, 0, 2, 3).reshape(C, 64, HM)
        if qh == 0:
            out[b][:, 0:64, :] = v
        else:
            out[b][:, 64:128, :] = v[:, ::-1, :]
    out += main_feature
    return out


# revision 32
# speedup vs baseline: 1.1263x; 1.0002x over previous
"""Trainium2 Bass kernel for nn_CrossLocal (cross-attention + convs + BN +
bilinear resizes), fused into a SINGLE launch across 8 NeuronCores.

Sharding: core = 2*b + qh owns half the query rows of batch b's 64x64 cross
image (32 rows = 2048 queries).  For qh=1 the row axis is mirrored on the
host so every core runs the identical SPMD program.  The single upsample
halo row each core needs (the partner's edge row) is computed on the host
(host prep already does the 1x1-conv projections) and shipped as an input,
so no halo ever rides a collective and no PE cycles are spent on it.

Host precomputes: y = theta_w@cr (keys), x = g_w@cr (queries),
z = phi(avgpool2x2(main)) row-scaled by e^r where r[m] = g_b.y_m is the
softmax-invariant bias, with an extra e^r column that accumulates the
softmax denominator for free.  Keys are PERMUTED: sorted by |y_m| and
interleaved low/high so that even-position key tiles (low score magnitude,
never softmax-dominant) can take an approximate exponential.

Device pipeline per core, per query superchunk J (1024 q) x key tile t:
  S[t] = y_t^T @ x_J                (2 bf16 matmuls, 4x PE row-tiling)
  es   = exp(S)                     split across TWO engines:
           odd tiles  -> ScalarE exact exp        (table spline)
           even tiles -> VectorE Schraudolph exp: int16(A*s+B) bitcast as
                         bf16 (~+-3% which is harmless on low-norm keys)
  acc += z_t^T @ es                 (K=128 bf16 matmuls into PSUM)
The PE matmul stream is the wall-clock: S pairs are emitted adjacently so
their row-quarter tiles overlap, and each az rides two tiles behind its
exp so the PE FIFO never head-of-line stalls on an in-flight exp (a >3.4us
PE bubble re-throttles the HAM clock gate to 1.2 GHz).

Per-J epilogue (h = (W@num + b*den)/den, BN stats) is cut into small steps
drained one-per-tile-pair inside the NEXT J's loop for the same reason.
BN statistics ride two tiny [64,2] AllGathers: AR#1 (J0 stats) overlaps
the J1 loop; AR#2 (J1 stats) is the only collective on the tail, hidden
behind the 2x bilinear upsample.  BN affine is folded into the output
epilogue (s*u + t + main), fp16 I/O throughout.
"""

import math
import os
from contextlib import ExitStack

import numpy as np
import ml_dtypes

import concourse.bass as bass
import concourse.tile as tile
from concourse import bacc, mybir
from concourse.bass_utils import run_bass_kernel_spmd

f32 = mybir.dt.float32
f32r = mybir.dt.float32r
fp16 = mybir.dt.float16
bf16 = mybir.dt.bfloat16
i16 = mybir.dt.int16
AF = mybir.ActivationFunctionType
ALU = mybir.AluOpType

NPBF16 = ml_dtypes.bfloat16

B, C, CI = 4, 64, 32
HM, HC = 128, 64
NC = HC * HC          # 4096 keys
NQ = 2048             # owned queries per core
NH = 64               # halo row width (host-computed)
NT = NC // 128        # 32 key tiles
BN_EPS = 1e-5
NSTAT = float(B * NC)
CORES = list(range(8))

# Schraudolph exp: es = bitcast_bf16(int16(EXPA*s + EXPB))
EXPA = 128.0 / math.log(2.0)
EXPB = 16256.0 - 0.058 * 128.0 + 0.5   # -c*128 minimax bias, +0.5 rounding

# engine split: tile t is VectorE-approx iff t even and t//2 < NDVE[J]
# (J1 gives VectorE one more tile: ScalarE's last exp paces the final az
# matmul and the AR#2 trigger chain)
NDVE = (16, 15)

_cache = {}
last_profile = {}
_run_counter = [0]


def _run(nc, in_maps, name):
    trace = os.environ.get("BASS_PROFILE", "") == "1"
    tmpdir = None
    if trace:
        _run_counter[0] += 1
        tmpdir = os.path.join("/tmp/bass_traces",
                              f"{name}_{_run_counter[0]}")
        os.makedirs(tmpdir, exist_ok=True)
    br = run_bass_kernel_spmd(
        nc, in_maps, core_ids=CORES, trace=trace, tmpdir=tmpdir
    )
    if trace:
        last_profile[name] = br
    return br.results


def _build():
    nc = bacc.Bacc("TRN2", target_bir_lowering=False, debug=False,
                   num_devices=len(CORES))
    d_y1 = nc.dram_tensor("y1", [32, NC], bf16, kind="ExternalInput").ap()
    d_x1 = nc.dram_tensor("x1", [32, NQ], bf16, kind="ExternalInput").ap()
    d_z = nc.dram_tensor("zaug", [128, NT * 33], bf16, kind="ExternalInput").ap()
    d_wwt = nc.dram_tensor("wwt", [33, C], f32, kind="ExternalInput").ap()
    d_ones = nc.dram_tensor("ones64", [1, C], f32, kind="ExternalInput").ap()
    d_hh = nc.dram_tensor("hhalo", [C, NH], fp16, kind="ExternalInput").ap()
    d_g16 = nc.dram_tensor("g16", [128, 1], f32, kind="ExternalInput").ap()
    d_bet = nc.dram_tensor("bet", [128, 1], f32, kind="ExternalInput").ap()
    d_o = nc.dram_tensor("outp", [128, 32 * HM], fp16, kind="ExternalOutput").ap()

    with ExitStack() as ctx:
        tc = ctx.enter_context(tile.TileContext(nc))
        const = ctx.enter_context(tc.tile_pool(name="const", bufs=1))
        work = ctx.enter_context(tc.tile_pool(name="work", bufs=1))
        es_s = ctx.enter_context(tc.tile_pool(name="es_s", bufs=5))
        ps_s = ctx.enter_context(tc.tile_pool(name="ps_s", bufs=3, space="PSUM"))
        ps_fix = ctx.enter_context(tc.tile_pool(name="ps_fix", bufs=1, space="PSUM"))
        dram = ctx.enter_context(tc.tile_pool(name="dram", bufs=1, space="DRAM"))

        # --- warm the exp table ASAP ---
        warm = const.tile([1, 1], f32, tag="warm")
        nc.vector.memset(warm, 0.0)
        nc.scalar.activation(out=warm, in_=warm, func=AF.Exp)

        wsrc = const.tile([128, 512], bf16, tag="wsrc")
        nc.vector.memset(wsrc, 1.0)

        # --- constant loads: y/x ship un-replicated, quarters are
        # replicated on-device by SBUF-to-SBUF DMAs (4x less upload).
        # Replicas are emitted in fine chunks right behind the upload
        # chunk they mirror, so quarter-j tiles unblock progressively
        # instead of waiting for one big replica at the end of the queue.
        x4 = const.tile([128, NQ], bf16, tag="x4")
        nc.sync.dma_start(out=x4[0:32, 0:1024], in_=d_x1[:, 0:1024])
        for q in range(1, 4):
            nc.sync.dma_start(out=x4[32 * q:32 * q + 32, 0:1024],
                              in_=x4[0:32, 0:1024])
        y4 = const.tile([128, NC], bf16, tag="y4")
        for yc in range(4):
            sl = slice(1024 * yc, 1024 * yc + 1024)
            nc.sync.dma_start(out=y4[0:32, sl], in_=d_y1[:, sl])
            for q in range(1, 4):
                nc.sync.dma_start(out=y4[32 * q:32 * q + 32, sl],
                                  in_=y4[0:32, sl])
        nc.gpsimd.dma_start(out=x4[0:32, 1024:NQ], in_=d_x1[:, 1024:NQ])
        for q in range(1, 4):
            nc.gpsimd.dma_start(out=x4[32 * q:32 * q + 32, 1024:NQ],
                                in_=x4[0:32, 1024:NQ])
        z_sb = const.tile([128, NT, 33], bf16, tag="z_sb")
        nc.gpsimd.dma_start(out=z_sb, in_=d_z.rearrange("p (t s) -> p t s", s=33))
        wwt = const.tile([33, C], f32r, tag="wwt")
        nc.gpsimd.dma_start(out=wwt, in_=d_wwt)
        ones64 = const.tile([1, C], f32r, tag="ones64")
        nc.gpsimd.dma_start(out=ones64, in_=d_ones)
        g16 = const.tile([128, 1], f32, tag="g16")
        nc.sync.dma_start(out=g16, in_=d_g16)
        bet = const.tile([128, 1], f32, tag="bet")
        nc.sync.dma_start(out=bet, in_=d_bet)

        # --- persistent tiles ---
        h = work.tile([C, NQ], fp16, tag="h")
        junk = work.tile([C, 1024], fp16, tag="junk")
        sh = work.tile([C, 2], f32, tag="sh")
        sq = work.tile([C, 2], f32, tag="sq")
        acc = ps_fix.tile([128, 1024], f32, tag="acc")
        h2 = work.tile([128, 18, 66], fp16, tag="h2")
        # host-computed halo row lands straight in its upsample slot
        nc.gpsimd.dma_start(
            out=h2[C:128, 17:18, 1:65],
            in_=d_hh.rearrange("c (r w) -> c r w", w=64),
        )

        # DRAM bounce buffers for the two stats AllGathers
        ag1_in = dram.tile([C, 2], f32)
        ag1_out = dram.tile([8 * C, 2], f32)
        ag2_in = dram.tile([C, 2], f32)
        ag2_out = dram.tile([8 * C, 2], f32)

        def ham_filler(n):
            for wi in range(n):
                nc.tensor.matmul(
                    out=acc[0:C, 0:512], lhsT=wsrc[:, 0:C], rhs=wsrc,
                    start=(wi == 0), stop=(wi == n - 1),
                    skip_group_check=True,
                )

        ham_filler(10)

        def epilogue_steps(qoff, jj):
            """Emit-closures computing h[:, qoff:qoff+1024] from the az
            accumulator acc[64jj:64jj+33] (matmul scratch in acc's
            partitions 0:64, dead while the other J accumulates)."""
            accp = acc[64 * jj:64 * jj + 33, :]
            dbc_s, hp_s = acc[0:C, 0:512], acc[0:C, 512:1024]
            nsb = work.tile([33, 1024], f32r, tag="nsb", bufs=2)
            steps = [lambda: nc.vector.tensor_copy(out=nsb, in_=accp)]

            def chunk_a(c):
                nc.tensor.matmul(
                    out=dbc_s, lhsT=ones64, rhs=nsb[0:1, c:c + 512],
                    start=True, stop=True, skip_group_check=True,
                )
                rec = work.tile([C, 512], f32, tag="rec", bufs=2)
                nc.vector.reciprocal_approx_fast(out=rec, in_=dbc_s)
                return rec

            def chunk_b(c, rec):
                nc.tensor.matmul(
                    out=hp_s, lhsT=wwt, rhs=nsb[:, c:c + 512],
                    start=True, stop=True, skip_group_check=True,
                )
                nc.vector.tensor_tensor(
                    out=h[:, qoff + c:qoff + c + 512],
                    in0=hp_s, in1=rec, op=ALU.mult,
                )

            box = {}
            skip = lambda: None
            for c in (0, 512):
                # a full loop-pair of padding between each PE-bearing step
                # and its DVE-produced input, so the PE FIFO never stalls
                steps.append(skip)
                steps.append(lambda c=c: box.__setitem__(c, chunk_a(c)))
                steps.append(skip)
                steps.append(lambda c=c: chunk_b(c, box[c]))
            hv = h[:, qoff:qoff + 1024]
            steps.append(lambda: nc.vector.tensor_reduce(
                out=sh[:, jj:jj + 1], in_=hv,
                axis=mybir.AxisListType.X, op=ALU.add,
            ))
            steps.append(lambda: nc.scalar.activation(
                out=junk, in_=hv, func=AF.Square,
                accum_out=sq[:, jj:jj + 1],
            ))
            return steps

        def stats_trigger(jj, ag_in, ag_out):
            pay = work.tile([C, 2], f32, tag=f"pay{jj}")
            nc.gpsimd.tensor_copy(out=pay[:, 0:1], in_=sh[:, jj:jj + 1])
            nc.gpsimd.tensor_copy(out=pay[:, 1:2], in_=sq[:, jj:jj + 1])
            nc.gpsimd.dma_start(out=ag_in, in_=pay)
            nc.gpsimd.collective_compute(
                "AllGather", ALU.bypass,
                replica_groups=[CORES],
                ins=[ag_in[:].opt()], outs=[ag_out[:].opt()],
            )

        def stats_readback(jj, ag_out):
            g = work.tile([128, 8, 2], f32, tag=f"g{jj}")
            gv = ag_out.rearrange("(r c) s -> c r s", c=C)
            nc.sync.dma_start(out=g[0:C], in_=gv)
            nc.sync.dma_start(out=g[C:128], in_=gv)
            red = work.tile([128, 2], f32, tag=f"red{jj}")
            nc.vector.tensor_reduce(
                out=red, in_=g.rearrange("c r s -> c s r"),
                axis=mybir.AxisListType.X, op=ALU.add,
            )
            return red

        # ================= main attention loop =================
        # one flat stream of 64 tiles: the az pipeline (lag one pair) flows
        # straight across the J0/J1 boundary, so the PE never sees the
        # boundary drain bubble that re-throttled the HAM to half clock
        pending = []
        es_q = {}

        def emit_az(J, t):
            es = es_q.pop((J, t))
            for c in range(2):
                nc.tensor.matmul(
                    out=acc[64 * J:64 * J + 33, 512 * c:512 * c + 512],
                    lhsT=z_sb[:, t, :],
                    rhs=es[:, 512 * c:512 * c + 512],
                    start=(t == 0), stop=(t == NT - 1),
                    skip_group_check=True,
                    tile_position=(0, 64 * J),
                )

        for gp in range(NT):
            J, p = gp // 16, (gp % 16) * 2
            qoff = 1024 * J
            # S pair back-to-back: distinct row quarters overlap on PE
            sps = {}
            for t in (p, p + 1):
                # quarters 1-3 are on-device replicas that land a few
                # us in; the first tiles stick to quarter 0
                j = 0 if gp < 4 else t % 4
                s_ps = ps_s.tile([128, 1024], f32, tag="s")
                sps[t] = s_ps
                for c in range(2):
                    nc.tensor.matmul(
                        out=s_ps[:, 512 * c:512 * c + 512],
                        lhsT=y4[32 * j:32 * j + 32, 128 * t:128 * t + 128],
                        rhs=x4[32 * j:32 * j + 32,
                               qoff + 512 * c:qoff + 512 * c + 512],
                        start=True, stop=True,
                        skip_group_check=True,
                        tile_position=(32 * j, 0),
                    )
            # the previous pair's az rides behind this pair's S: deps
            # satisfied, the PE FIFO never stalls on an in-flight exp
            if gp >= 1:
                pJ, pp = (gp - 1) // 16, ((gp - 1) % 16) * 2
                emit_az(pJ, pp)
                emit_az(pJ, pp + 1)
                if pJ == 0 and pp == NT - 2:
                    # J0's accumulator just closed: queue its epilogue
                    pending = epilogue_steps(0, 0)
                    pending.append(lambda: stats_trigger(0, ag1_in, ag1_out))
                    pending.append(lambda: nc.sync.dma_start(
                        out=h2[0:C, 1:17, 1:65],
                        in_=h[:, 0:1024].rearrange("c (r w) -> c r w", w=64)))
                    pending.append(lambda: nc.sync.dma_start(
                        out=h2[0:C, 0:1, 1:65],
                        in_=h[:, 0:64].rearrange("c (r w) -> c r w", w=64)))
            for t in (p, p + 1):
                approx = (t % 2 == 0) and (t // 2 < NDVE[J])
                es = es_s.tile([128, 1024], bf16, tag="es")
                es_q[(J, t)] = es
                if approx:
                    nc.vector.tensor_scalar(
                        out=es.bitcast(i16), in0=sps[t],
                        scalar1=EXPA, scalar2=EXPB,
                        op0=ALU.mult, op1=ALU.add,
                    )
                else:
                    nc.scalar.activation(out=es, in_=sps[t], func=AF.Exp)
            if pending:
                pending.pop(0)()
        emit_az(1, NT - 2)
        emit_az(1, NT - 1)
        for step in pending:
            step()
        for step in epilogue_steps(1024, 1):
            step()
        # warm the sqrt table before the BN affine needs it
        nc.scalar.activation(out=warm, in_=warm, func=AF.Sqrt)
        stats_trigger(1, ag2_in, ag2_out)

        # --- h2 rest: row 16 + rowgroup 1 rows 15..31 ---
        nc.sync.dma_start(
            out=h2[0:C, 17:18, 1:65],
            in_=h[:, 1024:1088].rearrange("c (r w) -> c r w", w=64),
        )
        nc.sync.dma_start(
            out=h2[C:128, 0:17, 1:65],
            in_=h[:, 15 * 64:NQ].rearrange("c (r w) -> c r w", w=64),
        )
        # column clamp pads
        nc.vector.tensor_copy(out=h2[:, :, 0:1], in_=h2[:, :, 1:2])
        nc.vector.tensor_copy(out=h2[:, :, 65:66], in_=h2[:, :, 64:65])

        # --- 2x bilinear upsample (x16 scale folded into BN affine) ---
        rp = work.tile([128, 32, 66], fp16, tag="rp")
        nc.vector.scalar_tensor_tensor(
            out=rp[:, 0::2, :], in0=h2[:, 1:17, :], scalar=3.0,
            in1=h2[:, 0:16, :], op0=ALU.mult, op1=ALU.add,
        )
        nc.vector.scalar_tensor_tensor(
            out=rp[:, 1::2, :], in0=h2[:, 1:17, :], scalar=3.0,
            in1=h2[:, 2:18, :], op0=ALU.mult, op1=ALU.add,
        )
        u = work.tile([128, 32, HM], fp16, tag="u")
        nc.vector.scalar_tensor_tensor(
            out=u[:, :, 0::2], in0=rp[:, :, 1:65], scalar=3.0,
            in1=rp[:, :, 0:64], op0=ALU.mult, op1=ALU.add,
        )
        nc.vector.scalar_tensor_tensor(
            out=u[:, :, 1::2], in0=rp[:, :, 1:65], scalar=3.0,
            in1=rp[:, :, 2:66], op0=ALU.mult, op1=ALU.add,
        )

        # --- global BN stats -> affine (post AR#2) ---
        # late-hint everything AllGather-dependent so the scheduler keeps
        # it out of the busy mid-kernel engine FIFOs
        late = ctx.enter_context(tc.tile_wait_until(0.085))
        red1 = stats_readback(0, ag1_out)
        red2 = stats_readback(1, ag2_out)
        red = work.tile([128, 2], f32, tag="red")
        nc.vector.tensor_tensor(out=red, in0=red1, in1=red2, op=ALU.add)
        mom = work.tile([128, 2], f32, tag="mom")
        nc.vector.tensor_scalar(
            out=mom, in0=red, scalar1=1.0 / NSTAT, scalar2=None, op0=ALU.mult
        )
        msq = work.tile([128, 1], f32, tag="msq")
        nc.vector.tensor_tensor(
            out=msq, in0=mom[:, 0:1], in1=mom[:, 0:1], op=ALU.mult
        )
        varep = work.tile([128, 1], f32, tag="varep")
        nc.vector.scalar_tensor_tensor(
            out=varep, in0=mom[:, 1:2], scalar=1.0, in1=msq,
            op0=ALU.mult, op1=ALU.subtract,
        )
        nc.vector.tensor_scalar(
            out=varep, in0=varep, scalar1=BN_EPS, scalar2=None, op0=ALU.add
        )
        sd = work.tile([128, 1], f32, tag="sd")
        nc.scalar.activation(out=sd, in_=varep, func=AF.Sqrt)
        isd = work.tile([128, 1], f32, tag="isd")
        nc.vector.reciprocal(out=isd, in_=sd)
        s2 = work.tile([128, 1], f32, tag="s2")
        nc.vector.tensor_tensor(out=s2, in0=isd, in1=g16, op=ALU.mult)
        mt = work.tile([128, 1], f32, tag="mt")
        nc.vector.scalar_tensor_tensor(
            out=mt, in0=mom[:, 0:1], scalar=16.0, in1=s2,
            op0=ALU.mult, op1=ALU.mult,
        )
        t2 = work.tile([128, 1], f32, tag="t2")
        nc.vector.tensor_tensor(out=t2, in0=bet, in1=mt, op=ALU.subtract)

        # --- out = s*u + t (residual +main added on the host) ---
        bnu = work.tile([128, 32 * HM], fp16, tag="bnu")
        HF = 16 * HM
        for part in range(2):
            sl = slice(part * HF, part * HF + HF)
            nc.vector.tensor_scalar(
                out=bnu[:, sl], in0=u.rearrange("p r w -> p (r w)")[:, sl],
                scalar1=s2, scalar2=t2, op0=ALU.mult, op1=ALU.add,
            )
            nc.sync.dma_start(out=d_o[:, sl], in_=bnu[:, sl])

    nc.compile()
    return nc


def _tile_order():
    """Key-tile layout: tiles alternate low-norm (even slots) / high-norm
    (odd slots) so the VectorE-approx tiles interleave with ScalarE tiles
    in time.  Returns perm[t] = sorted-tile-rank placed at position t."""
    order = np.empty(NT, dtype=np.int64)
    order[0::2] = np.arange(16)          # low-norm halves
    order[1::2] = 16 + np.arange(16)     # high-norm halves
    return order


def _host_prep(main_feature, cross_feature, g_w, g_b, theta_w, theta_b,
               phi_w, phi_b, w_w, w_b, bn_gamma, bn_beta):
    per_batch = []
    tile_order = _tile_order()
    for b in range(B):
        cr = cross_feature[b].reshape(C, NC).astype(np.float64)
        y = theta_w.astype(np.float64) @ cr                 # [32, 4096]
        x = g_w.astype(np.float64) @ cr                     # [32, 4096]
        r = (g_b.astype(np.float64) @ y)                    # [4096]
        m = main_feature[b]
        pm = 0.25 * (m[:, 0::2, 0::2] + m[:, 0::2, 1::2]
                     + m[:, 1::2, 0::2] + m[:, 1::2, 1::2])
        z = phi_w.astype(np.float64) @ pm.reshape(C, NC).astype(np.float64)
        z = z + phi_b.astype(np.float64)[:, None]           # [32, 4096]

        # the two upsample halo rows (globals 31, 32) computed exactly
        xh = x.reshape(CI, HC, HC)[:, 31:33, :].reshape(CI, 2 * HC)
        sh_ = y.T @ xh + r[:, None]                         # [4096, 128]
        sh_ -= sh_.max(axis=0)
        esh = np.exp(sh_)
        att = (z @ esh) / esh.sum(axis=0)                   # [32, 128]
        hh = (w_w.astype(np.float64) @ att
              + w_b.astype(np.float64)[:, None])            # [64, 128]
        hh = hh.reshape(C, 2, HC).astype(np.float16)

        # permute keys: sort by |y| then interleave low/high norm tiles
        key_rank = np.argsort(np.linalg.norm(y, axis=0), kind="stable")
        key_perm = key_rank.reshape(NT, 128)[tile_order].reshape(-1)
        y = y[:, key_perm]
        z = z[:, key_perm]
        r = r[key_perm]

        y1 = y.astype(np.float32).astype(NPBF16)
        er = np.exp(r)
        zt = np.empty((128, NT, 33), dtype=np.float64)
        zt[:, :, 0] = 1.0
        zt[:, :, 1:] = z.T.reshape(NT, 128, CI).transpose(1, 0, 2)
        zt *= er.reshape(NT, 128).T[:, :, None]
        per_batch.append((x, y1, zt.astype(NPBF16), hh))

    wwt = np.concatenate([w_b[None, :], w_w.T], axis=0).astype(np.float32)
    ones64 = np.ones((1, C), dtype=np.float32)
    g16 = np.tile((bn_gamma / 16.0).astype(np.float32)[:, None], (2, 1))
    bet = np.tile(bn_beta.astype(np.float32)[:, None], (2, 1))

    in_maps = []
    for core in CORES:
        b, qh = core // 2, core % 2
        x, y1, zt, hh = per_batch[b]
        # local row l -> global row g: qh=0: g=l ; qh=1: g=63-l
        if qh == 0:
            rows = np.arange(32)
        else:
            rows = 63 - np.arange(32)
        xq = x.reshape(CI, HC, HC)[:, rows, :].reshape(CI, NQ)
        x1 = xq.astype(np.float32).astype(NPBF16)
        # halo: the row just past local row 31 (global 32 / 31)
        hhalo = np.ascontiguousarray(hh[:, 1 - qh, :])
        in_maps.append({
            "y1": y1, "x1": x1,
            "zaug": zt.reshape(128, NT * 33),
            "wwt": wwt, "ones64": ones64, "hhalo": hhalo,
            "g16": g16, "bet": bet,
        })
    return in_maps


def kernel(main_feature, cross_feature, g_w, g_b, theta_w, theta_b,
           phi_w, phi_b, w_w, w_b, bn_gamma, bn_beta):
    main_feature = np.ascontiguousarray(main_feature, dtype=np.float32)
    cross_feature = np.ascontiguousarray(cross_feature, dtype=np.float32)

    if "k" not in _cache:
        _cache["k"] = _build()

    in_maps = _host_prep(main_feature, cross_feature, g_w, g_b, theta_w,
                         theta_b, phi_w, phi_b, w_w, w_b, bn_gamma, bn_beta)
    res = _run(_cache["k"], in_maps, "k")

    out = np.empty((B, C, HM, HM), dtype=np.float32)
    for core in CORES:
        b, qh = core // 2, core % 2
        v = res[core]["outp"].astype(np.float32).reshape(2, C, 32, HM)
        v = v.transpose(1, 0, 2, 3).reshape(C, 64, HM)
        if qh == 0:
            out[b][:, 0:64, :] = v
        else:
            out[b][:, 64:128, :] = v[:, ::-1, :]
    out += main_feature
    return out
